# revision 34
# baseline (speedup 1.0000x reference)
"""Trainium2 Bass kernel for a 5-layer GIN graph-property model.

Structure exploited (from the problem's generator):
  - 5000 graphs x 20 nodes each; every edge is intra-graph, so message
    passing is block-diagonal per graph.
  - Only the per-graph center node (local idx 0) reaches the head, so
    layer l's output is only needed on the backward receptive field
    D_l (D_4 = centers, D_{l-1} = D_l u in(D_l)).  On this data that
    is ~27% of all (node, layer) pairs -> the GIN MLPs (the dominant
    PE cost) run on packed column sets instead of all nodes.
  - Layer 0: agg0 = [(A+I)@OH | F9] @ [emb0; ecat0], so with
    TW = [emb0; ecat0] @ W1[0] folded on the host, layer 0 needs no
    message matmul, no one-hot embed and no W1 — just integer count
    features M133 (exact in bf16) times a table.
  - Eval-BN and the self-loop edge constant fold into W2/b1.

Sharding: graphs are sorted by receptive-field size and dealt
round-robin to 8 cores, so one SPMD program (shapes are compile-time
literals = per-position maxima over cores) fits all cores with ~3%
padding.  Zero collectives.  The program is built lazily on first
kernel() call from the actual input's packing plan.
"""

import sys

import numpy as np
import ml_dtypes

from contextlib import ExitStack

try:
    from concourse import bass, bacc, tile, masks
except ImportError:
    for _p in ("/opt/trn_rl_repo", "/root/.axon_site/_ro/trn_rl_repo"):
        if _p not in sys.path:
            sys.path.append(_p)
    from concourse import bass, bacc, tile, masks
import concourse.mybir as mybir

BF16 = mybir.dt.bfloat16
F32 = mybir.dt.float32
AF = mybir.ActivationFunctionType

# static problem config
L, D, T = 5, 512, 12
G, NPG = 5000, 20
N, E = G * NPG, 200000
NCORES = 8
GPC = G // NCORES          # 625 graphs per core
NPC = GPC * NPG            # 12500 nodes per core
TILE_G = 25                # graph positions per tile
NT = GPC // TILE_G         # 25 tiles per core
ROWCAP = 119               # block row capacity; rows 119:128 hold ecat
GCOL = 500                 # MLP group width (psum bank = 512 f32)
EPS = 1e-5
SELF_LOOP_BOND = 4

_bf16 = ml_dtypes.bfloat16


def _build_program(plan, reps=1):
    """plan: dict with per-layer packed layout literals (see _make_plan).

    reps > 1 chains the full compute body back-to-back (layer 0 restarts
    from the DMAed count features, so every rep recomputes the identical
    result).  Used by the timing harness to amortize per-launch runtime
    overhead out of the measurement."""
    off = plan["off"]          # off[l][p] col offset of position p, l=0..4
    P = plan["P"]              # P[l] total packed cols per core
    blocks = plan["blocks"]    # blocks[l][t] = ((p0,p1), ...) for l=1..4
    PB = sum(P[1:])            # total bd cols

    nc = bacc.Bacc(None)

    mhi = nc.declare_dram_parameter("mhi", [128, P[0]], BF16, isOutput=False)
    mlo = nc.declare_dram_parameter("mlo", [5, P[0]], BF16, isOutput=False)
    twhi = nc.declare_dram_parameter("twhi", [128, 8, 128], BF16, isOutput=False)
    twlo = nc.declare_dram_parameter("twlo", [5, 8, 128], BF16, isOutput=False)
    bdp = nc.declare_dram_parameter("bdp", [128, PB], BF16, isOutput=False)
    w1 = nc.declare_dram_parameter("w1", [L, 128, 4, 8, 128], BF16, isOutput=False)
    w2 = nc.declare_dram_parameter("w2", [L, 128, 8, 4, 128], BF16, isOutput=False)
    b1 = nc.declare_dram_parameter("b1", [L, 128, 8], F32, isOutput=False)
    b2 = nc.declare_dram_parameter("b2", [L, 128, 4], F32, isOutput=False)
    ecat = nc.declare_dram_parameter("ecat", [L, 9, D], BF16, isOutput=False)
    hw1 = nc.declare_dram_parameter("hw1", [128, 4, 128], BF16, isOutput=False)
    hw2 = nc.declare_dram_parameter("hw2", [128, T], BF16, isOutput=False)
    hb1 = nc.declare_dram_parameter("hb1", [128, 1], F32, isOutput=False)
    hb2 = nc.declare_dram_parameter("hb2", [T, 1], F32, isOutput=False)
    out = nc.declare_dram_parameter("out", [T, GPC], F32, isOutput=True)

    bmax = max(max(len(bl) for bl in blocks[l]) for l in range(1, 5))

    def tile_cols(l, t):
        return off[l][min((t + 1) * TILE_G, GPC)] - off[l][t * TILE_G]

    # groups[l] = list of (col0, gn, t_ready)
    groups = {}
    for l in range(5):
        gs = []
        for c0 in range(0, P[l], GCOL):
            gn = min(GCOL, P[l] - c0)
            # tile containing the group's last column
            t_ready = 0
            for t in range(NT):
                if off[l][min((t + 1) * TILE_G, GPC)] >= c0 + gn:
                    t_ready = t
                    break
            gs.append((c0, gn, t_ready))
        groups[l] = gs

    with tile.TileContext(nc) as tc, ExitStack() as ctx:
        const = ctx.enter_context(tc.tile_pool(name="const", bufs=1))
        hpool = ctx.enter_context(tc.tile_pool(name="h", bufs=1))
        wpool = ctx.enter_context(tc.tile_pool(name="w", bufs=2))
        io = ctx.enter_context(tc.tile_pool(name="io", bufs=3))
        work = ctx.enter_context(tc.tile_pool(name="work", bufs=2))
        psum = ctx.enter_context(tc.tile_pool(name="psum", bufs=3, space="PSUM"))
        psmlp = ctx.enter_context(tc.tile_pool(name="psmlp", bufs=1, space="PSUM"))
        pairp = ctx.enter_context(tc.tile_pool(name="pair", bufs=1))
        hnm_pool = ctx.enter_context(tc.tile_pool(name="hnm", bufs=2))

        ident = const.tile([128, 128], BF16)
        masks.make_identity(nc, ident[:])

        twhi_s = const.tile([128, 8, 128], BF16)
        nc.sync.dma_start(twhi_s[:], twhi[:])
        twlo_s = const.tile([5, 8, 128], BF16)
        nc.sync.dma_start(twlo_s[:], twlo[:])

        # packed node features, feature-major.  Layer l's packed output
        # columns are a prefix-shrinking layout (off[l] <= off[l-1]
        # pointwise), so every layer can write into the same buffer the
        # previous layer is being consumed from without clobbering unread
        # columns.  Layer 4 output = centers at cols [0, 625) = head input.
        hT = hpool.tile([128, 4, P[0]], BF16)

        def mlp_group(l, g, aggt, w1_t, w2_t, b1_t, b2_t, pfx=""):
            c0, gn, _ = groups[l][g]
            func = AF.Relu if l < 4 else AF.Identity
            hm = pairp.tile([128, 8, GCOL], BF16, tag=f"hmid{g % 5}",
                            name=f"hmid_l{l}g{g}{pfx}")
            for m2 in range(8):
                ps = psmlp.tile([128, gn], F32, tag=f"mlp{g % 5}",
                                name=f"psh_l{l}g{g}{pfx}")
                for k in range(4):
                    nc.tensor.matmul(ps[:], w1_t[:, k, m2, :],
                                     aggt[:, k, 0:gn],
                                     start=(k == 0), stop=(k == 3))
                nc.scalar.activation(hm[:, m2, 0:gn], ps[:], AF.Relu,
                                     bias=b1_t[:, m2:m2 + 1])
            for m3 in range(4):
                ps = psmlp.tile([128, gn], F32, tag=f"mlp{g % 5}",
                                name=f"pso_l{l}g{g}{pfx}")
                for k2 in range(8):
                    nc.tensor.matmul(ps[:], w2_t[:, k2, m3, :],
                                     hm[:, k2, 0:gn],
                                     start=(k2 == 0), stop=(k2 == 7))
                nc.scalar.activation(hT[:, m3, c0:c0 + gn], ps[:], func,
                                     bias=b2_t[:, m3:m3 + 1])

        # REPS>1 duplicates the whole compute (idempotent: layer 0 restarts
        # from the DMAed count features) — diagnostic for separating device
        # time from per-launch runtime overhead.
        for _rep in range(reps):
            _stageB(nc, tc, plan, groups, blocks, off, P, bmax, tile_cols,
                    mhi, mlo, bdp, w1, w2, b1, b2, ecat,
                    twhi_s, twlo_s, ident, hT,
                    wpool, io, psum, psmlp, pairp, hnm_pool, mlp_group,
                    pfx=f"r{_rep}")

        # ---- head on the packed centers hT[:, :, 0:625] ----
        hw1_s = const.tile([128, 4, 128], BF16)
        nc.sync.dma_start(hw1_s[:], hw1[:])
        hw2_s = const.tile([128, T], BF16)
        nc.sync.dma_start(hw2_s[:], hw2[:])
        hb1_s = const.tile([128, 1], F32)
        nc.sync.dma_start(hb1_s[:], hb1[:])
        hb2_s = const.tile([T, 1], F32)
        nc.sync.dma_start(hb2_s[:], hb2[:])

        zT = work.tile([128, GPC], BF16, tag="z")
        out_s = const.tile([T, GPC], F32)
        for g0, gn in ((0, 320), (320, 305)):
            ps_z = psmlp.tile([128, gn], F32, tag="mlp0")
            for k in range(4):
                nc.tensor.matmul(ps_z[:], hw1_s[:, k, :],
                                 hT[:, k, g0:g0 + gn],
                                 start=(k == 0), stop=(k == 3))
            nc.scalar.activation(zT[:, g0:g0 + gn], ps_z[:], AF.Relu,
                                 bias=hb1_s[:, 0:1])
            ps_y = psmlp.tile([T, gn], F32, tag="mlp0")
            nc.tensor.matmul(ps_y[:], hw2_s[:], zT[:, g0:g0 + gn],
                             start=True, stop=True)
            nc.scalar.activation(out_s[:, g0:g0 + gn], ps_y[:], AF.Identity,
                                 bias=hb2_s[:, 0:1])
        nc.sync.dma_start(out[:], out_s[:])

    nc.compile()
    _dedupe_ldweights(nc)
    return nc


def _stageB(nc, tc, plan, groups, blocks, off, P, bmax, tile_cols,
            mhi, mlo, bdp, w1, w2, b1, b2, ecat,
            twhi_s, twlo_s, ident, hT,
            wpool, io, psum, psmlp, pairp, hnm_pool, mlp_group, pfx=""):
        # ---- layer 0: hmid0 = relu(M133 @ TW + b1f) from count features ----
        w2_t = wpool.tile([128, 8, 4, 128], BF16, tag="w2")
        nc.sync.dma_start(w2_t[:], w2[0])
        b1_t = wpool.tile([128, 8], F32, tag="b1")
        nc.sync.dma_start(b1_t[:], b1[0])
        b2_t = wpool.tile([128, 4], F32, tag="b2")
        nc.sync.dma_start(b2_t[:], b2[0])

        g0list = groups[0]
        for gp in range(0, len(g0list), 5):
            batch = list(range(gp, min(gp + 5, len(g0list))))
            ms, hmids, gns = [], [], []
            for bi, g in enumerate(batch):
                c0, gn, _ = g0list[g]
                gns.append(gn)
                mhi_t = io.tile([128, GCOL], BF16, tag="bd")
                nc.sync.dma_start(mhi_t[:, 0:gn], mhi[:, c0:c0 + gn])
                mlo_t = io.tile([5, GCOL], BF16, tag="bd4")
                nc.sync.dma_start(mlo_t[:, 0:gn], mlo[:, c0:c0 + gn])
                ms.append((mhi_t, mlo_t))
                hm = pairp.tile([128, 8, GCOL], BF16, tag=f"hmid{bi}",
                                name=f"hmid_l0g{g}{pfx}")
                hmids.append(hm)
            for m2 in range(8):
                pss = [psmlp.tile([128, gns[bi], ], F32, tag=f"mlp{bi}",
                                  name=f"psh_l0g{batch[bi]}{pfx}")
                       for bi in range(len(batch))]
                for bi in range(len(batch)):
                    nc.tensor.matmul(pss[bi][:], twhi_s[:, m2, :],
                                     ms[bi][0][:, 0:gns[bi]],
                                     start=True, stop=False)
                for bi in range(len(batch)):
                    nc.tensor.matmul(pss[bi][:], twlo_s[:, m2, :],
                                     ms[bi][1][:, 0:gns[bi]],
                                     start=False, stop=True)
                for bi in range(len(batch)):
                    nc.scalar.activation(hmids[bi][:, m2, 0:gns[bi]], pss[bi][:],
                                         AF.Relu, bias=b1_t[:, m2:m2 + 1])
            for m3 in range(4):
                pso = [psmlp.tile([128, gns[bi]], F32, tag=f"mlp{bi}",
                                  name=f"pso_l0g{batch[bi]}{pfx}")
                       for bi in range(len(batch))]
                for k2 in range(8):
                    for bi in range(len(batch)):
                        nc.tensor.matmul(pso[bi][:], w2_t[:, k2, m3, :],
                                         hmids[bi][:, k2, 0:gns[bi]],
                                         start=(k2 == 0), stop=(k2 == 7))
                for bi, g in enumerate(batch):
                    c0, gn, _ = g0list[g]
                    nc.scalar.activation(hT[:, m3, c0:c0 + gn], pso[bi][:],
                                         AF.Relu, bias=b2_t[:, m3:m3 + 1])

        # ---- layers 1..4: packed message passing + packed MLP ----
        bd_base = 0
        for l in range(1, 5):
            w1_t = wpool.tile([128, 4, 8, 128], BF16, tag="w1")
            nc.sync.dma_start(w1_t[:], w1[l])
            w2_t = wpool.tile([128, 8, 4, 128], BF16, tag="w2")
            nc.sync.dma_start(w2_t[:], w2[l])
            b1_t = wpool.tile([128, 8], F32, tag="b1")
            nc.sync.dma_start(b1_t[:], b1[l])
            b2_t = wpool.tile([128, 4], F32, tag="b2")
            nc.sync.dma_start(b2_t[:], b2[l])

            h_nm = hnm_pool.tile([128, bmax, D], BF16, tag="hnm")
            # rows between a block's real sources and 119 may hold stale
            # data times a zero bd row: must be finite, so clear once
            nc.gpsimd.memset(h_nm[:], 0.0)
            for k in range(bmax):
                nc.sync.dma_start(h_nm[ROWCAP:128, k, :], ecat[l])

            aggts = {}
            gnext = 0
            for t in range(NT):
                tc0 = off[l][t * TILE_G]
                tcn = tile_cols(l, t)
                if tcn > 0:
                    bd_t = io.tile([128, GCOL], BF16, tag="bd")
                    nc.sync.dma_start(
                        bd_t[:, 0:tcn],
                        bdp[:, bd_base + tc0: bd_base + tc0 + tcn])
                    # node-major source blocks via PE transpose
                    for k, (p0, p1) in enumerate(blocks[l][t]):
                        cs, ce = off[l - 1][p0], off[l - 1][p1]
                        R = ce - cs
                        ps_tr = psum.tile([ROWCAP, D], BF16, tag="msg",
                                          name=f"tr_l{l}t{t}b{k}{pfx}")
                        for m in range(4):
                            nc.tensor.transpose(
                                ps_tr[0:R, m * 128:(m + 1) * 128],
                                hT[:, m, cs:ce], ident[:])
                        nc.vector.tensor_copy(h_nm[0:R, k, :], ps_tr[0:R, :])
                    # agg (feature-major) for the tile's packed dst columns
                    # k-outer: block k's matmuls directly follow its h_nm
                    # copy, so the in-order PE never stalls on a fresh copy
                    ps_ms = [psum.tile([128, tcn], F32, tag="msg",
                                       name=f"agg_l{l}t{t}m{m}{pfx}")
                             for m in range(4)]
                    for k, (p0, p1) in enumerate(blocks[l][t]):
                        bs, be = off[l][p0] - tc0, off[l][p1] - tc0
                        if be > bs:
                            for m in range(4):
                                nc.tensor.matmul(
                                    ps_ms[m][:, bs:be],
                                    h_nm[:, k, m * 128:(m + 1) * 128],
                                    bd_t[:, bs:be], start=True, stop=True)
                    for m in range(4):
                        ps_m = ps_ms[m]
                        # scatter to the overlapped MLP group tiles
                        pos = 0
                        while pos < tcn:
                            g = (tc0 + pos) // GCOL
                            c0g = g * GCOL
                            gn = groups[l][g][1]
                            s0 = tc0 + pos - c0g
                            n = min(gn - s0, tcn - pos)
                            if g not in aggts:
                                aggts[g] = pairp.tile(
                                    [128, 4, GCOL], BF16, tag=f"agg{g % 5}",
                                    name=f"agg_l{l}g{g}{pfx}")
                            nc.vector.tensor_copy(
                                aggts[g][:, m, s0:s0 + n],
                                ps_m[:, pos:pos + n])
                            pos += n
                while gnext < len(groups[l]) and groups[l][gnext][2] <= t:
                    mlp_group(l, gnext, aggts[gnext], w1_t, w2_t, b1_t, b2_t, pfx)
                    gnext += 1
            bd_base += P[l]


def _dedupe_ldweights(nc):
    """Delete Ldweights that reload the exact stationary already in the PE
    array (identical weights AP as the immediately-preceding load).  The
    paired Matmult then reuses the loaded weights.  Waits on a deleted load
    are migrated onto its Matmult."""
    fn = nc.m.functions[0]
    for blk in fn.blocks:
        il = blk.instructions
        if len(il) < 100:
            continue
        new, last_key, n_del = [], None, 0
        for i, ins in enumerate(il):
            if ins.opcode == "Ldweights":
                key = ins.concise().split(" in=", 1)[-1]
                if (key == last_key and not ins.has_update()
                        and i + 1 < len(il)
                        and il[i + 1].opcode == "Matmult"):
                    if ins.has_wait():
                        mm = il[i + 1]
                        si = mm.sync_info
                        si.on_wait = list(si.on_wait) + list(
                            ins.sync_info.on_wait)
                        mm.sync_info = si
                    n_del += 1
                    continue
                last_key = key
            new.append(ins)
        if n_del:
            blk.instructions = new


_NC_CACHE = {}


def _get_program(plan, reps=1):
    key = (reps, tuple(plan["P"]),
           tuple(tuple(o) for o in plan["off"]),
           tuple(tuple(tuple(b) for b in plan["blocks"][l])
                 for l in range(1, 5)))
    h = hash(key)
    if h not in _NC_CACHE:
        _NC_CACHE[h] = _build_program(plan, reps)
    return _NC_CACHE[h]


def _make_plan(shat):
    """shat[l][p]: max-over-cores packed size of position p at layer l."""
    off, P = [], []
    for l in range(5):
        o = np.concatenate([[0], np.cumsum(shat[l])]).astype(np.int64)
        off.append(tuple(int(v) for v in o))
        P.append(int(o[-1]))
    blocks = {}
    for l in range(1, 5):
        bl = []
        for t in range(NT):
            bs, p0 = [], t * TILE_G
            pe = min((t + 1) * TILE_G, GPC)
            p = p0
            while p < pe:
                q, rows = p, 0
                while q < pe and rows + shat[l - 1][q] <= ROWCAP:
                    rows += shat[l - 1][q]
                    q += 1
                assert q > p, (l, t, p, shat[l - 1][p])
                bs.append((p, q))
                p = q
            bl.append(tuple(bs))
        blocks[l] = bl
    return dict(off=off, P=P, blocks=blocks)


# ---------------------------------------------------------------------------
# Execution path: build the jitted SPMD executable once, keep device-placed
# inputs cached; repeat execution is pure dispatch+execute+fetch.
# ---------------------------------------------------------------------------

_RUNNERS = {}   # reps -> (jitted_fn, in_names, out_names, out_avals, mesh)
_PERM = None    # graph order permutation (set by _prepare_inputs)
_PLAN = None    # packing plan (set by _prepare_inputs)


def _get_runner(plan=None, reps=1):
    if reps in _RUNNERS:
        return _RUNNERS[reps]
    assert plan is not None, "first call must supply a plan"

    import jax
    from jax.sharding import Mesh, PartitionSpec
    from jax.experimental.shard_map import shard_map
    from concourse.bass2jax import (
        _bass_exec_p, partition_id_tensor, install_neuronx_cc_hook)

    nc = _get_program(plan, reps)
    install_neuronx_cc_hook()

    partition_name = (nc.partition_id_tensor.name
                      if nc.partition_id_tensor else None)
    in_names, out_names, out_avals = [], [], []
    for alloc in nc.m.functions[0].allocations:
        if not isinstance(alloc, mybir.MemoryLocationSet):
            continue
        name = alloc.memorylocations[0].name
        if alloc.kind == "ExternalInput":
            if name != partition_name:
                in_names.append(name)
        elif alloc.kind == "ExternalOutput":
            out_names.append(name)
            out_avals.append(jax.core.ShapedArray(
                tuple(alloc.tensor_shape), mybir.dt.np(alloc.dtype)))
    n_params = len(in_names)
    n_outs = len(out_avals)
    all_in_names = in_names + out_names + (
        [partition_name] if partition_name else [])

    def _body(*args):
        operands = list(args)
        if partition_name is not None:
            operands.append(partition_id_tensor())
        return tuple(_bass_exec_p.bind(
            *operands,
            out_avals=tuple(out_avals),
            in_names=tuple(all_in_names),
            out_names=tuple(out_names),
            lowering_input_output_aliases=(),
            sim_require_finite=True,
            sim_require_nnan=True,
            nc=nc,
        ))

    devices = jax.devices()[:NCORES]
    mesh = Mesh(np.asarray(devices), ("core",))
    jitted = jax.jit(
        shard_map(_body, mesh=mesh,
                  in_specs=(PartitionSpec("core"),) * (n_params + n_outs),
                  out_specs=(PartitionSpec("core"),) * n_outs,
                  check_rep=False),
        keep_unused=True,
    )
    _RUNNERS[reps] = (jitted, in_names, out_names, out_avals, mesh)
    return _RUNNERS[reps]


def place_inputs(in_maps, reps=1):
    """Concatenate per-core in_maps along axis 0 and place each input on its
    core (sharded along axis 0 of the concatenated array)."""
    import jax
    from jax.sharding import NamedSharding, PartitionSpec

    _, in_names, _, _, mesh = _get_runner(_PLAN, reps)
    sharding = NamedSharding(mesh, PartitionSpec("core"))
    dev_in = []
    for nm in in_names:
        host = np.concatenate([m[nm] for m in in_maps], axis=0)
        dev_in.append(jax.device_put(host, sharding))
    for a in dev_in:
        a.block_until_ready()
    return dev_in


def make_out_dummies(reps=1):
    import jax
    from jax.sharding import NamedSharding, PartitionSpec

    _, _, _, out_avals, mesh = _get_runner(_PLAN, reps)
    sharding = NamedSharding(mesh, PartitionSpec("core"))
    ds = [jax.device_put(
        np.zeros((NCORES * s.shape[0], *s.shape[1:]), s.dtype), sharding)
        for s in out_avals]
    for a in ds:
        a.block_until_ready()
    return ds


def launch(dev_in, dummies, reps=1):
    jitted, _, _, _, _ = _get_runner(_PLAN, reps)
    return jitted(*dev_in, *dummies)


def assemble(out_arrs):
    out = np.asarray(out_arrs[0]).reshape(NCORES, T, GPC)
    rs = np.arange(G)
    res = np.empty((G, T), np.float32)
    res[_PERM] = out[rs % NCORES, :, rs // NCORES]
    return res


def run_placed(dev_in, dummies=None):
    if dummies is None:
        dummies = make_out_dummies()
    return assemble(launch(dev_in, dummies))


def _prepare_inputs(x, edge_index, edge_attr, batch, num_graphs,
                    emb1, emb2, eemb1, eemb2, W1, b1, W2, b2, bn_g, bn_b,
                    hW1, hb1, hg, hbt, hW2, hb2):
    """Host-side restructuring: receptive-field packing, parameter folding,
    count features, block-diagonal message matrices, shard by graph."""
    global _PERM, _PLAN
    x = np.asarray(x)
    edge_index = np.asarray(edge_index)
    edge_attr = np.asarray(edge_attr)
    fp = lambda a: np.asarray(a, np.float32)
    emb1, emb2 = fp(emb1), fp(emb2)
    eemb1, eemb2 = fp(eemb1), fp(eemb2)
    W1, b1, W2, b2 = fp(W1), fp(b1), fp(W2), fp(b2)
    bn_g, bn_b = fp(bn_g), fp(bn_b)
    hW1, hb1, hg, hbt, hW2, hb2 = (fp(hW1), fp(hb1), fp(hg), fp(hbt),
                                   fp(hW2), fp(hb2))

    bn_inv = np.float32(1.0 / np.sqrt(1.0 + EPS))

    # fold eval-BN into second linear of each GIN MLP
    W2f = W2 * (bn_g * bn_inv)[:, None, :]
    b2f = b2 * (bn_g * bn_inv) + bn_b
    # fold per-layer self-loop constant through W1 into b1
    c = eemb1[:, SELF_LOOP_BOND, :] + eemb2[:, 0, :]            # [L, D]
    b1f = b1 + np.einsum('ld,ldm->lm', c, W1)                   # [L, 2D]

    ecat = np.concatenate([eemb1, eemb2], axis=1)               # [L, 9, D]
    emb0 = np.concatenate([emb1, emb2], axis=0)                 # [124, D]

    src0 = edge_index[0].astype(np.int64)
    dst0 = edge_index[1].astype(np.int64)

    # --- backward receptive fields on the ORIGINAL graph ids ---
    masksL = np.zeros((5, N), bool)
    m = masksL[4]
    m[0::NPG] = True
    for l in (4, 3, 2, 1):
        nm = masksL[l].copy()
        nm[src0[masksL[l][dst0]]] = True
        masksL[l - 1] = nm
    sizes = masksL.reshape(5, G, NPG).sum(2)                    # [5, G]

    # --- sorted round-robin graph placement across cores ---
    # lexicographic by per-layer receptive-field size: positions (= octets of
    # 8 graphs, one per core) get near-identical size profiles at EVERY
    # layer, minimizing the per-position-max padding (~4% vs ~14% for a
    # sum key)
    skey = (sizes[0] * (1 << 30) + sizes[1] * (1 << 20)
            + sizes[2] * (1 << 10) + sizes[3])
    order = np.argsort(-skey, kind="stable")                    # rank -> old g
    core_of = np.arange(G) % NCORES
    pos_of = np.arange(G) // NCORES
    # new node id for (rank r, local j)
    newbase = np.empty(G, np.int64)
    newbase[order] = core_of * NPC + pos_of * NPG
    newid = newbase[np.arange(N) // NPG] + np.arange(N) % NPG
    inv = np.argsort(newid)                                     # new -> old
    _PERM = order

    x_n = x[inv]
    src, dst = newid[src0], newid[dst0]
    maskn = masksL[:, inv]                                      # [5, N] new ids
    # sizes per (l, core, pos)
    s_lcp = maskn.reshape(5, NCORES, GPC, NPG).sum(3)           # [5, 8, 625]
    shat = s_lcp.max(1)                                         # [5, 625]
    plan = _make_plan([tuple(int(v) for v in shat[l]) for l in range(5)])
    off = [np.asarray(o, np.int64) for o in plan["off"]]
    P = plan["P"]

    # packed column index per (l, node): off[l][pos] + rank-in-graph
    ngid = np.arange(N) // NPG                                  # new graph id
    npos = ngid % GPC
    colpos = np.full((5, N), -1, np.int64)
    for l in range(5):
        rk = maskn[l].reshape(G, NPG).cumsum(1).reshape(N) - 1
        sel = maskn[l]
        colpos[l, sel] = off[l][npos[sel]] + rk[sel]

    # rowstart per (l, pos): block start offset in packed l-1 layout
    rowstart = np.zeros((5, GPC), np.int64)
    for l in range(1, 5):
        for t in range(NT):
            for (p0, p1) in plan["blocks"][l][t]:
                rowstart[l, p0:p1] = off[l - 1][p0]

    # F9[v, j] (new ids): incoming bond/direction counts
    F9 = (np.bincount(dst * 9 + edge_attr[:, 0], minlength=N * 9)
          + np.bincount(dst * 9 + 6 + edge_attr[:, 1], minlength=N * 9)
          ).astype(np.float32).reshape(N, 9)

    # layer-0 count features (new ids)
    atom, chir = x_n[:, 0].astype(np.int64), x_n[:, 1].astype(np.int64)
    M124 = (np.bincount(dst * 124 + atom[src], minlength=N * 124)
            + np.bincount(dst * 124 + 120 + chir[src], minlength=N * 124)
            ).astype(np.float32).reshape(N, 124)
    M124[np.arange(N), atom] += 1.0
    M124[np.arange(N), 120 + chir] += 1.0
    M133 = np.concatenate([M124, F9], axis=1)                   # [N, 133]
    TW = (np.concatenate([emb0, ecat[0]], axis=0).astype(np.float64)
          @ W1[0].astype(np.float64)).astype(np.float32)        # [133, 2D]
    twhi_h = np.ascontiguousarray(TW[:128].reshape(128, 8, 128)).astype(_bf16)
    twlo_h = np.ascontiguousarray(TW[128:].reshape(5, 8, 128)).astype(_bf16)

    # shared (replicated) tensors
    w1_h = np.ascontiguousarray(
        W1.reshape(L, 4, 128, 8, 128).transpose(0, 2, 1, 3, 4)).astype(_bf16)
    w2_h = np.ascontiguousarray(
        W2f.reshape(L, 8, 128, 4, 128).transpose(0, 2, 1, 3, 4)).astype(_bf16)
    b1_h = np.ascontiguousarray(b1f.reshape(L, 8, 128).transpose(0, 2, 1))
    b2_h = np.ascontiguousarray(b2f.reshape(L, 4, 128).transpose(0, 2, 1))
    ecat_h = ecat.astype(_bf16)
    hW1s = hW1[:D] + hW1[D:]                                     # [512, 128]
    hw1_h = np.ascontiguousarray(
        hW1s.reshape(4, 128, 128).transpose(1, 0, 2)).astype(_bf16)
    hw2_h = (hW2 * (hg * bn_inv)[:, None]).astype(_bf16)         # [128, T]
    hb2f = (hb2 + hbt @ hW2).reshape(T, 1).astype(np.float32)
    hb1_h = hb1.reshape(128, 1).astype(np.float32)

    # --- per-core bd (block-diagonal + F9 rows) and packed M133 ---
    PB = sum(P[1:])
    core_of_node = np.arange(N) // NPC
    in_maps = []
    for cidx in range(NCORES):
        bdp_c = np.zeros((128, PB), np.float32)
        base = 0
        emask_c = core_of_node[dst] == cidx
        for l in range(1, 5):
            sel = emask_c & maskn[l][dst]
            u, v = src[sel], dst[sel]
            rows = colpos[l - 1][u] - rowstart[l][npos[v]]
            cols = base + colpos[l][v]
            np.add.at(bdp_c, (rows, cols), 1.0)
            # self term
            vs = np.flatnonzero(maskn[l] & (core_of_node == cidx))
            rs = colpos[l - 1][vs] - rowstart[l][npos[vs]]
            cs = base + colpos[l][vs]
            bdp_c[rs, cs] += 1.0
            # F9 rows at partitions 119:128
            bdp_c[np.repeat(np.arange(ROWCAP, 128), len(vs)),
                  np.tile(cs, 9)] = F9[vs].T.reshape(-1)
            base += P[l]
        m133_c = np.zeros((133, P[0]), np.float32)
        vs0 = np.flatnonzero(maskn[0] & (core_of_node == cidx))
        m133_c[:, colpos[0][vs0]] = M133[vs0].T
        in_maps.append(dict(
            mhi=m133_c[:128].astype(_bf16), mlo=m133_c[128:].astype(_bf16),
            twhi=twhi_h, twlo=twlo_h,
            bdp=bdp_c.astype(_bf16),
            w1=w1_h, w2=w2_h, b1=b1_h, b2=b2_h,
            ecat=ecat_h,
            hw1=hw1_h, hw2=hw2_h, hb1=hb1_h, hb2=hb2f,
        ))
    _PLAN = plan
    return in_maps


def kernel(**inputs) -> np.ndarray:
    in_maps = _prepare_inputs(**inputs)
    _get_runner(_PLAN)
    dev_in = place_inputs(in_maps)
    return run_placed(dev_in)


# revision 45
# speedup vs baseline: 1.0066x; 1.0066x over previous
"""Trainium2 Bass kernel for a 5-layer GIN graph-property model.

Structure exploited (from the problem's generator):
  - 5000 graphs x 20 nodes each; every edge is intra-graph, so message
    passing is block-diagonal per graph.
  - Only the per-graph center node (local idx 0) reaches the head, so
    layer l's output is only needed on the backward receptive field
    D_l (D_4 = centers, D_{l-1} = D_l u in(D_l)).  On this data that
    is ~27% of all (node, layer) pairs -> the GIN MLPs (the dominant
    PE cost) run on packed column sets instead of all nodes.
  - Layer 0: agg0 = [(A+I)@OH | F9] @ [emb0; ecat0], so with
    TW = [emb0; ecat0] @ W1[0] folded on the host, layer 0 needs no
    message matmul, no one-hot embed and no W1 — just integer count
    features M133 (exact in bf16) times a table.
  - Eval-BN and the self-loop edge constant fold into W2/b1.

Sharding: graphs are sorted by receptive-field size and dealt
round-robin to 8 cores, so one SPMD program (shapes are compile-time
literals = per-position maxima over cores) fits all cores with ~3%
padding.  Zero collectives.  The program is built lazily on first
kernel() call from the actual input's packing plan.
"""

import sys

import numpy as np
import ml_dtypes

from contextlib import ExitStack

try:
    from concourse import bass, bacc, tile, masks
except ImportError:
    for _p in ("/opt/trn_rl_repo", "/root/.axon_site/_ro/trn_rl_repo"):
        if _p not in sys.path:
            sys.path.append(_p)
    from concourse import bass, bacc, tile, masks
import concourse.mybir as mybir

BF16 = mybir.dt.bfloat16
F32 = mybir.dt.float32
AF = mybir.ActivationFunctionType

# static problem config
L, D, T = 5, 512, 12
G, NPG = 5000, 20
N, E = G * NPG, 200000
NCORES = 8
GPC = G // NCORES          # 625 graphs per core
NPC = GPC * NPG            # 12500 nodes per core
TILE_G = 25                # graph positions per tile
NT = GPC // TILE_G         # 25 tiles per core
ROWCAP = 119               # block row capacity; rows 119:128 hold ecat
GCOL = 500                 # MLP group width (psum bank = 512 f32)
EPS = 1e-5
SELF_LOOP_BOND = 4

_bf16 = ml_dtypes.bfloat16


def _build_program(plan, reps=1):
    """plan: dict with per-layer packed layout literals (see _make_plan).

    reps > 1 chains the full compute body back-to-back (layer 0 restarts
    from the DMAed count features, so every rep recomputes the identical
    result).  Used by the timing harness to amortize per-launch runtime
    overhead out of the measurement."""
    off = plan["off"]          # off[l][p] col offset of position p, l=0..4
    P = plan["P"]              # P[l] total packed cols per core
    blocks = plan["blocks"]    # blocks[l][t] = ((p0,p1), ...) for l=1..4
    PB = sum(P[1:])            # total bd cols

    nc = bacc.Bacc(None)

    mhi = nc.declare_dram_parameter("mhi", [128, P[0]], BF16, isOutput=False)
    mlo = nc.declare_dram_parameter("mlo", [5, P[0]], BF16, isOutput=False)
    twhi = nc.declare_dram_parameter("twhi", [128, 8, 128], BF16, isOutput=False)
    twlo = nc.declare_dram_parameter("twlo", [5, 8, 128], BF16, isOutput=False)
    bdp = nc.declare_dram_parameter("bdp", [128, PB], BF16, isOutput=False)
    w1 = nc.declare_dram_parameter("w1", [L, 128, 4, 8, 128], BF16, isOutput=False)
    w2 = nc.declare_dram_parameter("w2", [L, 128, 8, 4, 128], BF16, isOutput=False)
    b1 = nc.declare_dram_parameter("b1", [L, 128, 8], F32, isOutput=False)
    b2 = nc.declare_dram_parameter("b2", [L, 128, 4], F32, isOutput=False)
    ecat = nc.declare_dram_parameter("ecat", [L, 9, D], BF16, isOutput=False)
    hw1 = nc.declare_dram_parameter("hw1", [128, 4, 128], BF16, isOutput=False)
    hw2 = nc.declare_dram_parameter("hw2", [128, T], BF16, isOutput=False)
    hb1 = nc.declare_dram_parameter("hb1", [128, 1], F32, isOutput=False)
    hb2 = nc.declare_dram_parameter("hb2", [T, 1], F32, isOutput=False)
    out = nc.declare_dram_parameter("out", [T, GPC], F32, isOutput=True)

    bmax = max(max(len(bl) for bl in blocks[l]) for l in range(1, 5))

    def tile_cols(l, t):
        return off[l][min((t + 1) * TILE_G, GPC)] - off[l][t * TILE_G]

    # groups[l] = list of (col0, gn, t_ready)
    groups = {}
    for l in range(5):
        gs = []
        for c0 in range(0, P[l], GCOL):
            gn = min(GCOL, P[l] - c0)
            # tile containing the group's last column
            t_ready = 0
            for t in range(NT):
                if off[l][min((t + 1) * TILE_G, GPC)] >= c0 + gn:
                    t_ready = t
                    break
            gs.append((c0, gn, t_ready))
        groups[l] = gs

    with tile.TileContext(nc) as tc, ExitStack() as ctx:
        const = ctx.enter_context(tc.tile_pool(name="const", bufs=1))
        hpool = ctx.enter_context(tc.tile_pool(name="h", bufs=1))
        wpool = ctx.enter_context(tc.tile_pool(name="w", bufs=2))
        io = ctx.enter_context(tc.tile_pool(name="io", bufs=3))
        work = ctx.enter_context(tc.tile_pool(name="work", bufs=2))
        psum = ctx.enter_context(tc.tile_pool(name="psum", bufs=3, space="PSUM"))
        psmlp = ctx.enter_context(tc.tile_pool(name="psmlp", bufs=1, space="PSUM"))
        pairp = ctx.enter_context(tc.tile_pool(name="pair", bufs=1))
        hnm_pool = ctx.enter_context(tc.tile_pool(name="hnm", bufs=2))

        ident = const.tile([128, 128], BF16)
        masks.make_identity(nc, ident[:])

        twhi_s = const.tile([128, 8, 128], BF16)
        nc.sync.dma_start(twhi_s[:], twhi[:])
        twlo_s = const.tile([5, 8, 128], BF16)
        nc.sync.dma_start(twlo_s[:], twlo[:])

        # packed node features, feature-major.  Layer l's packed output
        # columns are a prefix-shrinking layout (off[l] <= off[l-1]
        # pointwise), so every layer can write into the same buffer the
        # previous layer is being consumed from without clobbering unread
        # columns.  Layer 4 output = centers at cols [0, 625) = head input.
        hT = hpool.tile([128, 4, P[0]], BF16)

        def mlp_group(l, g, aggt, w1_t, w2_t, b1_t, b2_t, pfx=""):
            c0, gn, _ = groups[l][g]
            func = AF.Relu if l < 4 else AF.Identity
            hm = pairp.tile([128, 8, GCOL], BF16, tag=f"hmid{g % 5}",
                            name=f"hmid_l{l}g{g}{pfx}")
            for m2 in range(8):
                ps = psmlp.tile([128, gn], F32, tag=f"mlp{g % 5}",
                                name=f"psh_l{l}g{g}{pfx}")
                for k in range(4):
                    nc.tensor.matmul(ps[:], w1_t[:, k, m2, :],
                                     aggt[:, k, 0:gn],
                                     start=(k == 0), stop=(k == 3))
                nc.scalar.activation(hm[:, m2, 0:gn], ps[:], AF.Relu,
                                     bias=b1_t[:, m2:m2 + 1])
            for m3 in range(4):
                ps = psmlp.tile([128, gn], F32, tag=f"mlp{g % 5}",
                                name=f"pso_l{l}g{g}{pfx}")
                for k2 in range(8):
                    nc.tensor.matmul(ps[:], w2_t[:, k2, m3, :],
                                     hm[:, k2, 0:gn],
                                     start=(k2 == 0), stop=(k2 == 7))
                nc.scalar.activation(hT[:, m3, c0:c0 + gn], ps[:], func,
                                     bias=b2_t[:, m3:m3 + 1])

        # REPS>1 duplicates the whole compute (idempotent: layer 0 restarts
        # from the DMAed count features) — diagnostic for separating device
        # time from per-launch runtime overhead.
        for _rep in range(reps):
            _stageB(nc, tc, plan, groups, blocks, off, P, bmax, tile_cols,
                    mhi, mlo, bdp, w1, w2, b1, b2, ecat,
                    twhi_s, twlo_s, ident, hT,
                    wpool, io, psum, psmlp, pairp, hnm_pool, mlp_group,
                    pfx=f"r{_rep}")

        # ---- head on the packed centers hT[:, :, 0:625] ----
        hw1_s = const.tile([128, 4, 128], BF16)
        nc.sync.dma_start(hw1_s[:], hw1[:])
        hw2_s = const.tile([128, T], BF16)
        nc.sync.dma_start(hw2_s[:], hw2[:])
        hb1_s = const.tile([128, 1], F32)
        nc.sync.dma_start(hb1_s[:], hb1[:])
        hb2_s = const.tile([T, 1], F32)
        nc.sync.dma_start(hb2_s[:], hb2[:])

        zT = work.tile([128, GPC], BF16, tag="z")
        out_s = const.tile([T, GPC], F32)
        for g0, gn in ((0, 320), (320, 305)):
            ps_z = psmlp.tile([128, gn], F32, tag="mlp0")
            for k in range(4):
                nc.tensor.matmul(ps_z[:], hw1_s[:, k, :],
                                 hT[:, k, g0:g0 + gn],
                                 start=(k == 0), stop=(k == 3))
            nc.scalar.activation(zT[:, g0:g0 + gn], ps_z[:], AF.Relu,
                                 bias=hb1_s[:, 0:1])
            ps_y = psmlp.tile([T, gn], F32, tag="mlp0")
            nc.tensor.matmul(ps_y[:], hw2_s[:], zT[:, g0:g0 + gn],
                             start=True, stop=True)
            nc.scalar.activation(out_s[:, g0:g0 + gn], ps_y[:], AF.Identity,
                                 bias=hb2_s[:, 0:1])
        nc.sync.dma_start(out[:], out_s[:])

    nc.compile()
    _dedupe_ldweights(nc)
    return nc


def _stageB(nc, tc, plan, groups, blocks, off, P, bmax, tile_cols,
            mhi, mlo, bdp, w1, w2, b1, b2, ecat,
            twhi_s, twlo_s, ident, hT,
            wpool, io, psum, psmlp, pairp, hnm_pool, mlp_group, pfx=""):
        # ---- layer 0: hmid0 = relu(M133 @ TW + b1f) from count features ----
        w2_t = wpool.tile([128, 8, 4, 128], BF16, tag="w2")
        nc.sync.dma_start(w2_t[:], w2[0])
        b1_t = wpool.tile([128, 8], F32, tag="b1")
        nc.sync.dma_start(b1_t[:], b1[0])
        b2_t = wpool.tile([128, 4], F32, tag="b2")
        nc.sync.dma_start(b2_t[:], b2[0])

        g0list = groups[0]
        for gp in range(0, len(g0list), 5):
            batch = list(range(gp, min(gp + 5, len(g0list))))
            ms, hmids, gns = [], [], []
            for bi, g in enumerate(batch):
                c0, gn, _ = g0list[g]
                gns.append(gn)
                mhi_t = io.tile([128, GCOL], BF16, tag="bd")
                nc.sync.dma_start(mhi_t[:, 0:gn], mhi[:, c0:c0 + gn])
                mlo_t = io.tile([5, GCOL], BF16, tag="bd4")
                nc.sync.dma_start(mlo_t[:, 0:gn], mlo[:, c0:c0 + gn])
                ms.append((mhi_t, mlo_t))
                hm = pairp.tile([128, 8, GCOL], BF16, tag=f"hmid{bi}",
                                name=f"hmid_l0g{g}{pfx}")
                hmids.append(hm)
            for m2 in range(8):
                pss = [psmlp.tile([128, gns[bi], ], F32, tag=f"mlp{bi}",
                                  name=f"psh_l0g{batch[bi]}{pfx}")
                       for bi in range(len(batch))]
                for bi in range(len(batch)):
                    nc.tensor.matmul(pss[bi][:], twhi_s[:, m2, :],
                                     ms[bi][0][:, 0:gns[bi]],
                                     start=True, stop=False)
                for bi in range(len(batch)):
                    nc.tensor.matmul(pss[bi][:], twlo_s[:, m2, :],
                                     ms[bi][1][:, 0:gns[bi]],
                                     start=False, stop=True)
                for bi in range(len(batch)):
                    nc.scalar.activation(hmids[bi][:, m2, 0:gns[bi]], pss[bi][:],
                                         AF.Relu, bias=b1_t[:, m2:m2 + 1])
            for m3 in range(4):
                pso = [psmlp.tile([128, gns[bi]], F32, tag=f"mlp{bi}",
                                  name=f"pso_l0g{batch[bi]}{pfx}")
                       for bi in range(len(batch))]
                for k2 in range(8):
                    for bi in range(len(batch)):
                        nc.tensor.matmul(pso[bi][:], w2_t[:, k2, m3, :],
                                         hmids[bi][:, k2, 0:gns[bi]],
                                         start=(k2 == 0), stop=(k2 == 7))
                for bi, g in enumerate(batch):
                    c0, gn, _ = g0list[g]
                    nc.scalar.activation(hT[:, m3, c0:c0 + gn], pso[bi][:],
                                         AF.Relu, bias=b2_t[:, m3:m3 + 1])

        # ---- layers 1..4: packed message passing + packed MLP ----
        bd_base = 0
        for l in range(1, 5):
            w1_t = wpool.tile([128, 4, 8, 128], BF16, tag="w1")
            nc.sync.dma_start(w1_t[:], w1[l])
            w2_t = wpool.tile([128, 8, 4, 128], BF16, tag="w2")
            nc.sync.dma_start(w2_t[:], w2[l])
            b1_t = wpool.tile([128, 8], F32, tag="b1")
            nc.sync.dma_start(b1_t[:], b1[l])
            b2_t = wpool.tile([128, 4], F32, tag="b2")
            nc.sync.dma_start(b2_t[:], b2[l])

            h_nm = hnm_pool.tile([128, bmax, D], BF16, tag="hnm")
            # rows between a block's real sources and 119 may hold stale
            # data times a zero bd row: must be finite, so clear once
            nc.gpsimd.memset(h_nm[:], 0.0)
            for k in range(bmax):
                nc.sync.dma_start(h_nm[ROWCAP:128, k, :], ecat[l])

            aggts = {}
            gnext = 0
            for t in range(NT):
                tc0 = off[l][t * TILE_G]
                tcn = tile_cols(l, t)
                if tcn > 0:
                    bd_t = io.tile([128, GCOL], BF16, tag="bd")
                    nc.sync.dma_start(
                        bd_t[:, 0:tcn],
                        bdp[:, bd_base + tc0: bd_base + tc0 + tcn])
                    # per-block: transpose sources node-major, one K=128
                    # matmul per feature chunk into a per-block psum bank
                    # ([128, 4, bcols] <= 2KB since bcols <= ROWCAP), one
                    # scatter copy per block.  Blocks pipeline through the
                    # shared psum ring: PE runs block k+1's transposes while
                    # DVE drains block k.
                    # software-pipelined: block k's transposes issue before
                    # block k-1's matmuls, so the in-order PE fills the
                    # transpose->DVE-copy latency with useful work
                    def emit_tr(k, p0, p1):
                        cs, ce = off[l - 1][p0], off[l - 1][p1]
                        R = ce - cs
                        ps_tr = psum.tile([ROWCAP, D], BF16, tag="msg",
                                          name=f"tr_l{l}t{t}b{k}{pfx}")
                        for m in range(4):
                            nc.tensor.transpose(
                                ps_tr[0:R, m * 128:(m + 1) * 128],
                                hT[:, m, cs:ce], ident[:])
                        nc.vector.tensor_copy(h_nm[0:R, k, :], ps_tr[0:R, :])

                    def emit_mm(k, p0, p1):
                        bs, be = off[l][p0], off[l][p1]
                        bn = be - bs
                        if bn == 0:
                            return
                        ps_b = psum.tile([128, 4, bn], F32, tag="msg",
                                         name=f"agg_l{l}t{t}b{k}{pfx}")
                        for m in range(4):
                            nc.tensor.matmul(
                                ps_b[:, m, :],
                                h_nm[:, k, m * 128:(m + 1) * 128],
                                bd_t[:, bs - tc0:be - tc0],
                                start=True, stop=True)
                        # scatter to the overlapped MLP group tiles
                        pos = bs
                        while pos < be:
                            g = pos // GCOL
                            gn = groups[l][g][1]
                            s0 = pos - g * GCOL
                            n = min(gn - s0, be - pos)
                            if g not in aggts:
                                aggts[g] = pairp.tile(
                                    [128, 4, GCOL], BF16, tag=f"agg{g % 5}",
                                    name=f"agg_l{l}g{g}{pfx}")
                            nc.vector.tensor_copy(
                                aggts[g][:, :, s0:s0 + n],
                                ps_b[:, :, pos - bs:pos - bs + n])
                            pos += n

                    bl = blocks[l][t]
                    for k, (p0, p1) in enumerate(bl):
                        emit_tr(k, p0, p1)
                        if k > 1:
                            emit_mm(k - 2, *bl[k - 2])
                    for k in range(max(len(bl) - 2, 0), len(bl)):
                        emit_mm(k, *bl[k])
                while gnext < len(groups[l]) and groups[l][gnext][2] <= t:
                    mlp_group(l, gnext, aggts[gnext], w1_t, w2_t, b1_t, b2_t, pfx)
                    gnext += 1
            bd_base += P[l]


def _dedupe_ldweights(nc):
    """Delete Ldweights that reload the exact stationary already in the PE
    array (identical weights AP as the immediately-preceding load).  The
    paired Matmult then reuses the loaded weights.  Waits on a deleted load
    are migrated onto its Matmult."""
    fn = nc.m.functions[0]
    for blk in fn.blocks:
        il = blk.instructions
        if len(il) < 100:
            continue
        new, last_key, n_del = [], None, 0
        for i, ins in enumerate(il):
            if ins.opcode == "Ldweights":
                key = ins.concise().split(" in=", 1)[-1]
                if (key == last_key and not ins.has_update()
                        and i + 1 < len(il)
                        and il[i + 1].opcode == "Matmult"):
                    if ins.has_wait():
                        mm = il[i + 1]
                        si = mm.sync_info
                        si.on_wait = list(si.on_wait) + list(
                            ins.sync_info.on_wait)
                        mm.sync_info = si
                    n_del += 1
                    continue
                last_key = key
            new.append(ins)
        if n_del:
            blk.instructions = new


_NC_CACHE = {}


def _get_program(plan, reps=1):
    key = (reps, tuple(plan["P"]),
           tuple(tuple(o) for o in plan["off"]),
           tuple(tuple(tuple(b) for b in plan["blocks"][l])
                 for l in range(1, 5)))
    h = hash(key)
    if h not in _NC_CACHE:
        _NC_CACHE[h] = _build_program(plan, reps)
    return _NC_CACHE[h]


def _make_plan(shat):
    """shat[l][p]: max-over-cores packed size of position p at layer l."""
    off, P = [], []
    for l in range(5):
        o = np.concatenate([[0], np.cumsum(shat[l])]).astype(np.int64)
        off.append(tuple(int(v) for v in o))
        P.append(int(o[-1]))
    blocks = {}
    for l in range(1, 5):
        bl = []
        for t in range(NT):
            bs, p0 = [], t * TILE_G
            pe = min((t + 1) * TILE_G, GPC)
            p = p0
            while p < pe:
                q, rows = p, 0
                while q < pe and rows + shat[l - 1][q] <= ROWCAP:
                    rows += shat[l - 1][q]
                    q += 1
                assert q > p, (l, t, p, shat[l - 1][p])
                bs.append((p, q))
                p = q
            bl.append(tuple(bs))
        blocks[l] = bl
    return dict(off=off, P=P, blocks=blocks)


# ---------------------------------------------------------------------------
# Execution path: build the jitted SPMD executable once, keep device-placed
# inputs cached; repeat execution is pure dispatch+execute+fetch.
# ---------------------------------------------------------------------------

_RUNNERS = {}   # reps -> (jitted_fn, in_names, out_names, out_avals, mesh)
_PERM = None    # graph order permutation (set by _prepare_inputs)
_PLAN = None    # packing plan (set by _prepare_inputs)


def _get_runner(plan=None, reps=1):
    if reps in _RUNNERS:
        return _RUNNERS[reps]
    assert plan is not None, "first call must supply a plan"

    import jax
    from jax.sharding import Mesh, PartitionSpec
    from jax.experimental.shard_map import shard_map
    from concourse.bass2jax import (
        _bass_exec_p, partition_id_tensor, install_neuronx_cc_hook)

    nc = _get_program(plan, reps)
    install_neuronx_cc_hook()

    partition_name = (nc.partition_id_tensor.name
                      if nc.partition_id_tensor else None)
    in_names, out_names, out_avals = [], [], []
    for alloc in nc.m.functions[0].allocations:
        if not isinstance(alloc, mybir.MemoryLocationSet):
            continue
        name = alloc.memorylocations[0].name
        if alloc.kind == "ExternalInput":
            if name != partition_name:
                in_names.append(name)
        elif alloc.kind == "ExternalOutput":
            out_names.append(name)
            out_avals.append(jax.core.ShapedArray(
                tuple(alloc.tensor_shape), mybir.dt.np(alloc.dtype)))
    n_params = len(in_names)
    n_outs = len(out_avals)
    all_in_names = in_names + out_names + (
        [partition_name] if partition_name else [])

    def _body(*args):
        operands = list(args)
        if partition_name is not None:
            operands.append(partition_id_tensor())
        return tuple(_bass_exec_p.bind(
            *operands,
            out_avals=tuple(out_avals),
            in_names=tuple(all_in_names),
            out_names=tuple(out_names),
            lowering_input_output_aliases=(),
            sim_require_finite=True,
            sim_require_nnan=True,
            nc=nc,
        ))

    devices = jax.devices()[:NCORES]
    mesh = Mesh(np.asarray(devices), ("core",))
    jitted = jax.jit(
        shard_map(_body, mesh=mesh,
                  in_specs=(PartitionSpec("core"),) * (n_params + n_outs),
                  out_specs=(PartitionSpec("core"),) * n_outs,
                  check_rep=False),
        keep_unused=True,
    )
    _RUNNERS[reps] = (jitted, in_names, out_names, out_avals, mesh)
    return _RUNNERS[reps]


def place_inputs(in_maps, reps=1):
    """Concatenate per-core in_maps along axis 0 and place each input on its
    core (sharded along axis 0 of the concatenated array)."""
    import jax
    from jax.sharding import NamedSharding, PartitionSpec

    _, in_names, _, _, mesh = _get_runner(_PLAN, reps)
    sharding = NamedSharding(mesh, PartitionSpec("core"))
    dev_in = []
    for nm in in_names:
        host = np.concatenate([m[nm] for m in in_maps], axis=0)
        dev_in.append(jax.device_put(host, sharding))
    for a in dev_in:
        a.block_until_ready()
    return dev_in


def make_out_dummies(reps=1):
    import jax
    from jax.sharding import NamedSharding, PartitionSpec

    _, _, _, out_avals, mesh = _get_runner(_PLAN, reps)
    sharding = NamedSharding(mesh, PartitionSpec("core"))
    ds = [jax.device_put(
        np.zeros((NCORES * s.shape[0], *s.shape[1:]), s.dtype), sharding)
        for s in out_avals]
    for a in ds:
        a.block_until_ready()
    return ds


def launch(dev_in, dummies, reps=1):
    jitted, _, _, _, _ = _get_runner(_PLAN, reps)
    return jitted(*dev_in, *dummies)


def assemble(out_arrs):
    out = np.asarray(out_arrs[0]).reshape(NCORES, T, GPC)
    rs = np.arange(G)
    res = np.empty((G, T), np.float32)
    res[_PERM] = out[rs % NCORES, :, rs // NCORES]
    return res


def run_placed(dev_in, dummies=None):
    if dummies is None:
        dummies = make_out_dummies()
    return assemble(launch(dev_in, dummies))


def _prepare_inputs(x, edge_index, edge_attr, batch, num_graphs,
                    emb1, emb2, eemb1, eemb2, W1, b1, W2, b2, bn_g, bn_b,
                    hW1, hb1, hg, hbt, hW2, hb2):
    """Host-side restructuring: receptive-field packing, parameter folding,
    count features, block-diagonal message matrices, shard by graph."""
    global _PERM, _PLAN
    x = np.asarray(x)
    edge_index = np.asarray(edge_index)
    edge_attr = np.asarray(edge_attr)
    fp = lambda a: np.asarray(a, np.float32)
    emb1, emb2 = fp(emb1), fp(emb2)
    eemb1, eemb2 = fp(eemb1), fp(eemb2)
    W1, b1, W2, b2 = fp(W1), fp(b1), fp(W2), fp(b2)
    bn_g, bn_b = fp(bn_g), fp(bn_b)
    hW1, hb1, hg, hbt, hW2, hb2 = (fp(hW1), fp(hb1), fp(hg), fp(hbt),
                                   fp(hW2), fp(hb2))

    bn_inv = np.float32(1.0 / np.sqrt(1.0 + EPS))

    # fold eval-BN into second linear of each GIN MLP
    W2f = W2 * (bn_g * bn_inv)[:, None, :]
    b2f = b2 * (bn_g * bn_inv) + bn_b
    # fold per-layer self-loop constant through W1 into b1
    c = eemb1[:, SELF_LOOP_BOND, :] + eemb2[:, 0, :]            # [L, D]
    b1f = b1 + np.einsum('ld,ldm->lm', c, W1)                   # [L, 2D]

    ecat = np.concatenate([eemb1, eemb2], axis=1)               # [L, 9, D]
    emb0 = np.concatenate([emb1, emb2], axis=0)                 # [124, D]

    src0 = edge_index[0].astype(np.int64)
    dst0 = edge_index[1].astype(np.int64)

    # --- backward receptive fields on the ORIGINAL graph ids ---
    masksL = np.zeros((5, N), bool)
    m = masksL[4]
    m[0::NPG] = True
    for l in (4, 3, 2, 1):
        nm = masksL[l].copy()
        nm[src0[masksL[l][dst0]]] = True
        masksL[l - 1] = nm
    sizes = masksL.reshape(5, G, NPG).sum(2)                    # [5, G]

    # --- sorted round-robin graph placement across cores ---
    # lexicographic by per-layer receptive-field size: positions (= octets of
    # 8 graphs, one per core) get near-identical size profiles at EVERY
    # layer, minimizing the per-position-max padding (~4% vs ~14% for a
    # sum key)
    skey = (sizes[0] * (1 << 30) + sizes[1] * (1 << 20)
            + sizes[2] * (1 << 10) + sizes[3])
    order = np.argsort(-skey, kind="stable")                    # rank -> old g
    core_of = np.arange(G) % NCORES
    pos_of = np.arange(G) // NCORES
    # new node id for (rank r, local j)
    newbase = np.empty(G, np.int64)
    newbase[order] = core_of * NPC + pos_of * NPG
    newid = newbase[np.arange(N) // NPG] + np.arange(N) % NPG
    inv = np.argsort(newid)                                     # new -> old
    _PERM = order

    x_n = x[inv]
    src, dst = newid[src0], newid[dst0]
    maskn = masksL[:, inv]                                      # [5, N] new ids
    # sizes per (l, core, pos)
    s_lcp = maskn.reshape(5, NCORES, GPC, NPG).sum(3)           # [5, 8, 625]
    shat = s_lcp.max(1)                                         # [5, 625]
    plan = _make_plan([tuple(int(v) for v in shat[l]) for l in range(5)])
    off = [np.asarray(o, np.int64) for o in plan["off"]]
    P = plan["P"]

    # packed column index per (l, node): off[l][pos] + rank-in-graph
    ngid = np.arange(N) // NPG                                  # new graph id
    npos = ngid % GPC
    colpos = np.full((5, N), -1, np.int64)
    for l in range(5):
        rk = maskn[l].reshape(G, NPG).cumsum(1).reshape(N) - 1
        sel = maskn[l]
        colpos[l, sel] = off[l][npos[sel]] + rk[sel]

    # rowstart per (l, pos): block start offset in packed l-1 layout
    rowstart = np.zeros((5, GPC), np.int64)
    for l in range(1, 5):
        for t in range(NT):
            for (p0, p1) in plan["blocks"][l][t]:
                rowstart[l, p0:p1] = off[l - 1][p0]

    # F9[v, j] (new ids): incoming bond/direction counts
    F9 = (np.bincount(dst * 9 + edge_attr[:, 0], minlength=N * 9)
          + np.bincount(dst * 9 + 6 + edge_attr[:, 1], minlength=N * 9)
          ).astype(np.float32).reshape(N, 9)

    # layer-0 count features (new ids)
    atom, chir = x_n[:, 0].astype(np.int64), x_n[:, 1].astype(np.int64)
    M124 = (np.bincount(dst * 124 + atom[src], minlength=N * 124)
            + np.bincount(dst * 124 + 120 + chir[src], minlength=N * 124)
            ).astype(np.float32).reshape(N, 124)
    M124[np.arange(N), atom] += 1.0
    M124[np.arange(N), 120 + chir] += 1.0
    M133 = np.concatenate([M124, F9], axis=1)                   # [N, 133]
    TW = (np.concatenate([emb0, ecat[0]], axis=0).astype(np.float64)
          @ W1[0].astype(np.float64)).astype(np.float32)        # [133, 2D]
    twhi_h = np.ascontiguousarray(TW[:128].reshape(128, 8, 128)).astype(_bf16)
    twlo_h = np.ascontiguousarray(TW[128:].reshape(5, 8, 128)).astype(_bf16)

    # shared (replicated) tensors
    w1_h = np.ascontiguousarray(
        W1.reshape(L, 4, 128, 8, 128).transpose(0, 2, 1, 3, 4)).astype(_bf16)
    w2_h = np.ascontiguousarray(
        W2f.reshape(L, 8, 128, 4, 128).transpose(0, 2, 1, 3, 4)).astype(_bf16)
    b1_h = np.ascontiguousarray(b1f.reshape(L, 8, 128).transpose(0, 2, 1))
    b2_h = np.ascontiguousarray(b2f.reshape(L, 4, 128).transpose(0, 2, 1))
    ecat_h = ecat.astype(_bf16)
    hW1s = hW1[:D] + hW1[D:]                                     # [512, 128]
    hw1_h = np.ascontiguousarray(
        hW1s.reshape(4, 128, 128).transpose(1, 0, 2)).astype(_bf16)
    hw2_h = (hW2 * (hg * bn_inv)[:, None]).astype(_bf16)         # [128, T]
    hb2f = (hb2 + hbt @ hW2).reshape(T, 1).astype(np.float32)
    hb1_h = hb1.reshape(128, 1).astype(np.float32)

    # --- per-core bd (block-diagonal + F9 rows) and packed M133 ---
    PB = sum(P[1:])
    core_of_node = np.arange(N) // NPC
    in_maps = []
    for cidx in range(NCORES):
        bdp_c = np.zeros((128, PB), np.float32)
        base = 0
        emask_c = core_of_node[dst] == cidx
        for l in range(1, 5):
            sel = emask_c & maskn[l][dst]
            u, v = src[sel], dst[sel]
            rows = colpos[l - 1][u] - rowstart[l][npos[v]]
            cols = base + colpos[l][v]
            np.add.at(bdp_c, (rows, cols), 1.0)
            # self term
            vs = np.flatnonzero(maskn[l] & (core_of_node == cidx))
            rs = colpos[l - 1][vs] - rowstart[l][npos[vs]]
            cs = base + colpos[l][vs]
            bdp_c[rs, cs] += 1.0
            # F9 rows at partitions 119:128
            bdp_c[np.repeat(np.arange(ROWCAP, 128), len(vs)),
                  np.tile(cs, 9)] = F9[vs].T.reshape(-1)
            base += P[l]
        m133_c = np.zeros((133, P[0]), np.float32)
        vs0 = np.flatnonzero(maskn[0] & (core_of_node == cidx))
        m133_c[:, colpos[0][vs0]] = M133[vs0].T
        in_maps.append(dict(
            mhi=m133_c[:128].astype(_bf16), mlo=m133_c[128:].astype(_bf16),
            twhi=twhi_h, twlo=twlo_h,
            bdp=bdp_c.astype(_bf16),
            w1=w1_h, w2=w2_h, b1=b1_h, b2=b2_h,
            ecat=ecat_h,
            hw1=hw1_h, hw2=hw2_h, hb1=hb1_h, hb2=hb2f,
        ))
    _PLAN = plan
    return in_maps


def kernel(**inputs) -> np.ndarray:
    in_maps = _prepare_inputs(**inputs)
    _get_runner(_PLAN)
    dev_in = place_inputs(in_maps)
    return run_placed(dev_in)


# revision 53
# speedup vs baseline: 1.0764x; 1.0694x over previous
"""Trainium2 Bass kernel for a 5-layer GIN graph-property model.

Structure exploited (from the problem's generator):
  - 5000 graphs x 20 nodes each; every edge is intra-graph, so message
    passing is block-diagonal per graph.
  - Only the per-graph center node (local idx 0) reaches the head, so
    layer l's output is only needed on the backward receptive field
    D_l (D_4 = centers, D_{l-1} = D_l u in(D_l)).  On this data that
    is ~27% of all (node, layer) pairs -> the GIN MLPs (the dominant
    PE cost) run on packed column sets instead of all nodes.
  - Layer 0: agg0 = [(A+I)@OH | F9] @ [emb0; ecat0], so with
    TW = [emb0; ecat0] @ W1[0] folded on the host, layer 0 needs no
    message matmul, no one-hot embed and no W1 — just integer count
    features M133 (exact in bf16) times a table.
  - Eval-BN and the self-loop edge constant fold into W2/b1.

Sharding: graphs are sorted by receptive-field size and dealt
round-robin to 8 cores, so one SPMD program (shapes are compile-time
literals = per-position maxima over cores) fits all cores with ~3%
padding.  Zero collectives.  The program is built lazily on first
kernel() call from the actual input's packing plan.
"""

import sys

import numpy as np
import ml_dtypes

from contextlib import ExitStack

try:
    from concourse import bass, bacc, tile, masks
except ImportError:
    for _p in ("/opt/trn_rl_repo", "/root/.axon_site/_ro/trn_rl_repo"):
        if _p not in sys.path:
            sys.path.append(_p)
    from concourse import bass, bacc, tile, masks
import concourse.mybir as mybir

BF16 = mybir.dt.bfloat16
F32 = mybir.dt.float32
AF = mybir.ActivationFunctionType

# static problem config
L, D, T = 5, 512, 12
G, NPG = 5000, 20
N, E = G * NPG, 200000
NCORES = 8
GPC = G // NCORES          # 625 graphs per core
NPC = GPC * NPG            # 12500 nodes per core
TILE_G = 25                # graph positions per tile
NT = GPC // TILE_G         # 25 tiles per core
ROWCAP = 119               # block row capacity; rows 119:128 hold ecat
GCOL = 500                 # MLP group width (psum bank = 512 f32)
EPS = 1e-5
SELF_LOOP_BOND = 4

_bf16 = ml_dtypes.bfloat16


def _build_program(plan, reps=1):
    """plan: dict with per-layer packed layout literals (see _make_plan).

    reps > 1 chains the full compute body back-to-back (layer 0 restarts
    from the DMAed count features, so every rep recomputes the identical
    result).  Used by the timing harness to amortize per-launch runtime
    overhead out of the measurement."""
    off = plan["off"]          # off[l][p] col offset of position p, l=0..4
    P = plan["P"]              # P[l] total packed cols per core
    blocks = plan["blocks"]    # blocks[l][t] = ((p0,p1), ...) for l=1..4
    PB = sum(P[1:])            # total bd cols

    nc = bacc.Bacc(None)

    mhi = nc.declare_dram_parameter("mhi", [128, P[0]], BF16, isOutput=False)
    mlo = nc.declare_dram_parameter("mlo", [5, P[0]], BF16, isOutput=False)
    twhi = nc.declare_dram_parameter("twhi", [128, 8, 128], BF16, isOutput=False)
    twlo = nc.declare_dram_parameter("twlo", [5, 8, 128], BF16, isOutput=False)
    bdp = nc.declare_dram_parameter("bdp", [128, PB], BF16, isOutput=False)
    w1 = nc.declare_dram_parameter("w1", [L, 128, 4, 8, 128], BF16, isOutput=False)
    w2 = nc.declare_dram_parameter("w2", [L, 128, 8, 4, 128], BF16, isOutput=False)
    b1 = nc.declare_dram_parameter("b1", [L, 128, 8], F32, isOutput=False)
    b2 = nc.declare_dram_parameter("b2", [L, 128, 4], F32, isOutput=False)
    ecat = nc.declare_dram_parameter("ecat", [L, 9, D], BF16, isOutput=False)
    hw1 = nc.declare_dram_parameter("hw1", [128, 4, 128], BF16, isOutput=False)
    hw2 = nc.declare_dram_parameter("hw2", [128, T], BF16, isOutput=False)
    hb1 = nc.declare_dram_parameter("hb1", [128, 1], F32, isOutput=False)
    hb2 = nc.declare_dram_parameter("hb2", [T, 1], F32, isOutput=False)
    out = nc.declare_dram_parameter("out", [T, GPC], F32, isOutput=True)

    bmax = max(max(len(bl) for bl in blocks[l]) for l in range(1, 5))

    def tile_cols(l, t):
        return off[l][min((t + 1) * TILE_G, GPC)] - off[l][t * TILE_G]

    # groups[l] = list of (col0, gn, t_ready)
    groups = {}
    for l in range(5):
        gs = []
        for c0 in range(0, P[l], GCOL):
            gn = min(GCOL, P[l] - c0)
            # tile containing the group's last column
            t_ready = 0
            for t in range(NT):
                if off[l][min((t + 1) * TILE_G, GPC)] >= c0 + gn:
                    t_ready = t
                    break
            gs.append((c0, gn, t_ready))
        groups[l] = gs

    with tile.TileContext(nc) as tc, ExitStack() as ctx:
        const = ctx.enter_context(tc.tile_pool(name="const", bufs=1))
        hpool = ctx.enter_context(tc.tile_pool(name="h", bufs=1))
        wpool = ctx.enter_context(tc.tile_pool(name="w", bufs=2))
        io = ctx.enter_context(tc.tile_pool(name="io", bufs=3))
        work = ctx.enter_context(tc.tile_pool(name="work", bufs=2))
        psum = ctx.enter_context(tc.tile_pool(name="psum", bufs=3, space="PSUM"))
        psmlp = ctx.enter_context(tc.tile_pool(name="psmlp", bufs=1, space="PSUM"))
        pairp = ctx.enter_context(tc.tile_pool(name="pair", bufs=1))
        hnm_pool = ctx.enter_context(tc.tile_pool(name="hnm", bufs=2))

        ident = const.tile([128, 128], BF16)
        masks.make_identity(nc, ident[:])

        twhi_s = const.tile([128, 8, 128], BF16)
        nc.sync.dma_start(twhi_s[:], twhi[:])
        twlo_s = const.tile([5, 8, 128], BF16)
        nc.sync.dma_start(twlo_s[:], twlo[:])

        # packed node features, feature-major.  Layer l's packed output
        # columns are a prefix-shrinking layout (off[l] <= off[l-1]
        # pointwise), so every layer can write into the same buffer the
        # previous layer is being consumed from without clobbering unread
        # columns.  Layer 4 output = centers at cols [0, 625) = head input.
        hT = hpool.tile([128, 4, P[0]], BF16)

        def mlp_group(l, g, aggt, w1_t, w2_t, b1_t, b2_t, pfx=""):
            c0, gn, _ = groups[l][g]
            func = AF.Relu if l < 4 else AF.Identity
            hm = pairp.tile([128, 8, GCOL], BF16, tag=f"hmid{g % 5}",
                            name=f"hmid_l{l}g{g}{pfx}")
            for m2 in range(8):
                ps = psmlp.tile([128, gn], F32, tag=f"mlp{g % 5}",
                                name=f"psh_l{l}g{g}{pfx}")
                for k in range(4):
                    nc.tensor.matmul(ps[:], w1_t[:, k, m2, :],
                                     aggt[:, k, 0:gn],
                                     start=(k == 0), stop=(k == 3))
                nc.scalar.activation(hm[:, m2, 0:gn], ps[:], AF.Relu,
                                     bias=b1_t[:, m2:m2 + 1])
            for m3 in range(4):
                ps = psmlp.tile([128, gn], F32, tag=f"mlp{g % 5}",
                                name=f"pso_l{l}g{g}{pfx}")
                for k2 in range(8):
                    nc.tensor.matmul(ps[:], w2_t[:, k2, m3, :],
                                     hm[:, k2, 0:gn],
                                     start=(k2 == 0), stop=(k2 == 7))
                nc.scalar.activation(hT[:, m3, c0:c0 + gn], ps[:], func,
                                     bias=b2_t[:, m3:m3 + 1])

        # REPS>1 duplicates the whole compute (idempotent: layer 0 restarts
        # from the DMAed count features) — diagnostic for separating device
        # time from per-launch runtime overhead.
        for _rep in range(reps):
            _stageB(nc, tc, plan, groups, blocks, off, P, bmax, tile_cols,
                    mhi, mlo, bdp, w1, w2, b1, b2, ecat,
                    twhi_s, twlo_s, ident, hT,
                    wpool, io, psum, psmlp, pairp, hnm_pool, mlp_group,
                    pfx=f"r{_rep}")

        # ---- head on the packed centers hT[:, :, 0:625] ----
        hw1_s = const.tile([128, 4, 128], BF16)
        nc.sync.dma_start(hw1_s[:], hw1[:])
        hw2_s = const.tile([128, T], BF16)
        nc.sync.dma_start(hw2_s[:], hw2[:])
        hb1_s = const.tile([128, 1], F32)
        nc.sync.dma_start(hb1_s[:], hb1[:])
        hb2_s = const.tile([T, 1], F32)
        nc.sync.dma_start(hb2_s[:], hb2[:])

        zT = work.tile([128, GPC], BF16, tag="z")
        out_s = const.tile([T, GPC], F32)
        for g0, gn in ((0, 320), (320, 305)):
            ps_z = psmlp.tile([128, gn], F32, tag="mlp0")
            for k in range(4):
                nc.tensor.matmul(ps_z[:], hw1_s[:, k, :],
                                 hT[:, k, g0:g0 + gn],
                                 start=(k == 0), stop=(k == 3))
            nc.scalar.activation(zT[:, g0:g0 + gn], ps_z[:], AF.Relu,
                                 bias=hb1_s[:, 0:1])
            ps_y = psmlp.tile([T, gn], F32, tag="mlp0")
            nc.tensor.matmul(ps_y[:], hw2_s[:], zT[:, g0:g0 + gn],
                             start=True, stop=True)
            nc.scalar.activation(out_s[:, g0:g0 + gn], ps_y[:], AF.Identity,
                                 bias=hb2_s[:, 0:1])
        nc.sync.dma_start(out[:], out_s[:])

    nc.compile()
    _dedupe_ldweights(nc)
    return nc


def _stageB(nc, tc, plan, groups, blocks, off, P, bmax, tile_cols,
            mhi, mlo, bdp, w1, w2, b1, b2, ecat,
            twhi_s, twlo_s, ident, hT,
            wpool, io, psum, psmlp, pairp, hnm_pool, mlp_group, pfx=""):
        # ---- layer 0: hmid0 = relu(M133 @ TW + b1f) from count features ----
        w2_t = wpool.tile([128, 8, 4, 128], BF16, tag="w2")
        nc.sync.dma_start(w2_t[:], w2[0])
        b1_t = wpool.tile([128, 8], F32, tag="b1")
        nc.sync.dma_start(b1_t[:], b1[0])
        b2_t = wpool.tile([128, 4], F32, tag="b2")
        nc.sync.dma_start(b2_t[:], b2[0])

        g0list = groups[0]
        for gp in range(0, len(g0list), 5):
            batch = list(range(gp, min(gp + 5, len(g0list))))
            ms, hmids, gns = [], [], []
            for bi, g in enumerate(batch):
                c0, gn, _ = g0list[g]
                gns.append(gn)
                mhi_t = io.tile([128, GCOL], BF16, tag="bd")
                nc.sync.dma_start(mhi_t[:, 0:gn], mhi[:, c0:c0 + gn])
                mlo_t = io.tile([5, GCOL], BF16, tag="bd4")
                nc.sync.dma_start(mlo_t[:, 0:gn], mlo[:, c0:c0 + gn])
                ms.append((mhi_t, mlo_t))
                hm = pairp.tile([128, 8, GCOL], BF16, tag=f"hmid{bi}",
                                name=f"hmid_l0g{g}{pfx}")
                hmids.append(hm)
            for m2 in range(8):
                pss = [psmlp.tile([128, gns[bi], ], F32, tag=f"mlp{bi}",
                                  name=f"psh_l0g{batch[bi]}{pfx}")
                       for bi in range(len(batch))]
                for bi in range(len(batch)):
                    nc.tensor.matmul(pss[bi][:], twhi_s[:, m2, :],
                                     ms[bi][0][:, 0:gns[bi]],
                                     start=True, stop=False)
                for bi in range(len(batch)):
                    nc.tensor.matmul(pss[bi][:], twlo_s[:, m2, :],
                                     ms[bi][1][:, 0:gns[bi]],
                                     start=False, stop=True)
                for bi in range(len(batch)):
                    nc.scalar.activation(hmids[bi][:, m2, 0:gns[bi]], pss[bi][:],
                                         AF.Relu, bias=b1_t[:, m2:m2 + 1])
            for m3 in range(4):
                pso = [psmlp.tile([128, gns[bi]], F32, tag=f"mlp{bi}",
                                  name=f"pso_l0g{batch[bi]}{pfx}")
                       for bi in range(len(batch))]
                for k2 in range(8):
                    for bi in range(len(batch)):
                        nc.tensor.matmul(pso[bi][:], w2_t[:, k2, m3, :],
                                         hmids[bi][:, k2, 0:gns[bi]],
                                         start=(k2 == 0), stop=(k2 == 7))
                for bi, g in enumerate(batch):
                    c0, gn, _ = g0list[g]
                    nc.scalar.activation(hT[:, m3, c0:c0 + gn], pso[bi][:],
                                         AF.Relu, bias=b2_t[:, m3:m3 + 1])

        # ---- layers 1..4: packed message passing + packed MLP ----
        bd_base = 0
        for l in range(1, 5):
            w1_t = wpool.tile([128, 4, 8, 128], BF16, tag="w1")
            nc.sync.dma_start(w1_t[:], w1[l])
            w2_t = wpool.tile([128, 8, 4, 128], BF16, tag="w2")
            nc.sync.dma_start(w2_t[:], w2[l])
            b1_t = wpool.tile([128, 8], F32, tag="b1")
            nc.sync.dma_start(b1_t[:], b1[l])
            b2_t = wpool.tile([128, 4], F32, tag="b2")
            nc.sync.dma_start(b2_t[:], b2[l])

            h_nm = hnm_pool.tile([128, bmax, D], BF16, tag="hnm")
            # rows between a block's real sources and 119 may hold stale
            # data times a zero bd row: must be finite, so clear once
            nc.gpsimd.memset(h_nm[:], 0.0)
            for k in range(bmax):
                nc.sync.dma_start(h_nm[ROWCAP:128, k, :], ecat[l])

            aggts = {}
            gnext = 0
            for t in range(NT):
                tc0 = off[l][t * TILE_G]
                tcn = tile_cols(l, t)
                if tcn > 0:
                    bd_t = io.tile([128, GCOL], BF16, tag="bd")
                    nc.sync.dma_start(
                        bd_t[:, 0:tcn],
                        bdp[:, bd_base + tc0: bd_base + tc0 + tcn])
                    # per-block: transpose sources node-major, one K=128
                    # matmul per feature chunk into a per-block psum bank
                    # ([128, 4, bcols] <= 2KB since bcols <= ROWCAP), one
                    # scatter copy per block.  Blocks pipeline through the
                    # shared psum ring: PE runs block k+1's transposes while
                    # DVE drains block k.
                    # software-pipelined: block k's transposes issue before
                    # block k-1's matmuls, so the in-order PE fills the
                    # transpose->DVE-copy latency with useful work
                    def emit_tr(k, p0, p1):
                        cs, ce = off[l - 1][p0], off[l - 1][p1]
                        R = ce - cs
                        ps_tr = psum.tile([ROWCAP, D], BF16, tag="msg",
                                          name=f"tr_l{l}t{t}b{k}{pfx}")
                        for m in range(4):
                            nc.tensor.transpose(
                                ps_tr[0:R, m * 128:(m + 1) * 128],
                                hT[:, m, cs:ce], ident[:])
                        nc.vector.tensor_copy(h_nm[0:R, k, :], ps_tr[0:R, :])

                    def emit_mm(k, p0, p1):
                        bs, be = off[l][p0], off[l][p1]
                        bn = be - bs
                        if bn == 0:
                            return
                        ps_b = psum.tile([128, 4, bn], F32, tag="msg",
                                         name=f"agg_l{l}t{t}b{k}{pfx}")
                        for m in range(4):
                            nc.tensor.matmul(
                                ps_b[:, m, :],
                                h_nm[:, k, m * 128:(m + 1) * 128],
                                bd_t[:, bs - tc0:be - tc0],
                                start=True, stop=True)
                        # scatter to the overlapped MLP group tiles
                        pos = bs
                        while pos < be:
                            g = pos // GCOL
                            gn = groups[l][g][1]
                            s0 = pos - g * GCOL
                            n = min(gn - s0, be - pos)
                            if g not in aggts:
                                aggts[g] = pairp.tile(
                                    [128, 4, GCOL], BF16, tag=f"agg{g % 5}",
                                    name=f"agg_l{l}g{g}{pfx}")
                            nc.vector.tensor_copy(
                                aggts[g][:, :, s0:s0 + n],
                                ps_b[:, :, pos - bs:pos - bs + n])
                            pos += n

                    bl = blocks[l][t]
                    for k, (p0, p1) in enumerate(bl):
                        emit_tr(k, p0, p1)
                        if k > 1:
                            emit_mm(k - 2, *bl[k - 2])
                    for k in range(max(len(bl) - 2, 0), len(bl)):
                        emit_mm(k, *bl[k])
                while gnext < len(groups[l]) and groups[l][gnext][2] <= t:
                    mlp_group(l, gnext, aggts[gnext], w1_t, w2_t, b1_t, b2_t, pfx)
                    gnext += 1
            bd_base += P[l]


def _dedupe_ldweights(nc):
    """Delete Ldweights that reload the exact stationary already in the PE
    array (identical weights AP as the immediately-preceding load).  The
    paired Matmult then reuses the loaded weights.  Waits on a deleted load
    are migrated onto its Matmult."""
    fn = nc.m.functions[0]
    for blk in fn.blocks:
        il = blk.instructions
        if len(il) < 100:
            continue
        new, last_key, n_del = [], None, 0
        for i, ins in enumerate(il):
            if ins.opcode == "Ldweights":
                key = ins.concise().split(" in=", 1)[-1]
                if (key == last_key and not ins.has_update()
                        and i + 1 < len(il)
                        and il[i + 1].opcode == "Matmult"):
                    if ins.has_wait():
                        mm = il[i + 1]
                        si = mm.sync_info
                        si.on_wait = list(si.on_wait) + list(
                            ins.sync_info.on_wait)
                        mm.sync_info = si
                    n_del += 1
                    continue
                last_key = key
            new.append(ins)
        if n_del:
            blk.instructions = new


_NC_CACHE = {}


def _get_program(plan, reps=1):
    key = (reps, tuple(plan["P"]),
           tuple(tuple(o) for o in plan["off"]),
           tuple(tuple(tuple(b) for b in plan["blocks"][l])
                 for l in range(1, 5)))
    h = hash(key)
    if h not in _NC_CACHE:
        _NC_CACHE[h] = _build_program(plan, reps)
    return _NC_CACHE[h]


def _make_plan(shat):
    """shat[l][p]: max-over-cores packed size of position p at layer l."""
    off, P = [], []
    for l in range(5):
        o = np.concatenate([[0], np.cumsum(shat[l])]).astype(np.int64)
        off.append(tuple(int(v) for v in o))
        P.append(int(o[-1]))
    blocks = {}
    for l in range(1, 5):
        bl = []
        for t in range(NT):
            bs, p0 = [], t * TILE_G
            pe = min((t + 1) * TILE_G, GPC)
            p = p0
            while p < pe:
                q, rows = p, 0
                while q < pe and rows + shat[l - 1][q] <= ROWCAP:
                    rows += shat[l - 1][q]
                    q += 1
                assert q > p, (l, t, p, shat[l - 1][p])
                bs.append((p, q))
                p = q
            bl.append(tuple(bs))
        blocks[l] = bl
    return dict(off=off, P=P, blocks=blocks)


# ---------------------------------------------------------------------------
# Execution path: build the jitted SPMD executable once, keep device-placed
# inputs cached; repeat execution is pure dispatch+execute+fetch.
# ---------------------------------------------------------------------------

_RUNNERS = {}   # reps -> (jitted_fn, in_names, out_names, out_avals, mesh)
_PERM = None    # graph order permutation (set by _prepare_inputs)
_PLAN = None    # packing plan (set by _prepare_inputs)


def _get_runner(plan=None, reps=1):
    if reps in _RUNNERS:
        return _RUNNERS[reps]
    assert plan is not None, "first call must supply a plan"

    import jax
    from jax.sharding import Mesh, PartitionSpec
    from jax.experimental.shard_map import shard_map
    from concourse.bass2jax import (
        _bass_exec_p, partition_id_tensor, install_neuronx_cc_hook)

    nc = _get_program(plan, reps)
    install_neuronx_cc_hook()

    partition_name = (nc.partition_id_tensor.name
                      if nc.partition_id_tensor else None)
    in_names, out_names, out_avals = [], [], []
    for alloc in nc.m.functions[0].allocations:
        if not isinstance(alloc, mybir.MemoryLocationSet):
            continue
        name = alloc.memorylocations[0].name
        if alloc.kind == "ExternalInput":
            if name != partition_name:
                in_names.append(name)
        elif alloc.kind == "ExternalOutput":
            out_names.append(name)
            out_avals.append(jax.core.ShapedArray(
                tuple(alloc.tensor_shape), mybir.dt.np(alloc.dtype)))
    n_params = len(in_names)
    n_outs = len(out_avals)
    all_in_names = in_names + out_names + (
        [partition_name] if partition_name else [])

    def _body(*args):
        operands = list(args)
        if partition_name is not None:
            operands.append(partition_id_tensor())
        return tuple(_bass_exec_p.bind(
            *operands,
            out_avals=tuple(out_avals),
            in_names=tuple(all_in_names),
            out_names=tuple(out_names),
            lowering_input_output_aliases=(),
            sim_require_finite=True,
            sim_require_nnan=True,
            nc=nc,
        ))

    devices = jax.devices()[:NCORES]
    mesh = Mesh(np.asarray(devices), ("core",))
    jitted = jax.jit(
        shard_map(_body, mesh=mesh,
                  in_specs=(PartitionSpec("core"),) * (n_params + n_outs),
                  out_specs=(PartitionSpec("core"),) * n_outs,
                  check_rep=False),
        keep_unused=True,
    )
    _RUNNERS[reps] = (jitted, in_names, out_names, out_avals, mesh)
    return _RUNNERS[reps]


def place_inputs(in_maps, reps=1):
    """Concatenate per-core in_maps along axis 0 and place each input on its
    core (sharded along axis 0 of the concatenated array)."""
    import jax
    from jax.sharding import NamedSharding, PartitionSpec

    _, in_names, _, _, mesh = _get_runner(_PLAN, reps)
    sharding = NamedSharding(mesh, PartitionSpec("core"))
    dev_in = []
    for nm in in_names:
        host = np.concatenate([m[nm] for m in in_maps], axis=0)
        dev_in.append(jax.device_put(host, sharding))
    for a in dev_in:
        a.block_until_ready()
    return dev_in


def make_out_dummies(reps=1):
    import jax
    from jax.sharding import NamedSharding, PartitionSpec

    _, _, _, out_avals, mesh = _get_runner(_PLAN, reps)
    sharding = NamedSharding(mesh, PartitionSpec("core"))
    ds = [jax.device_put(
        np.zeros((NCORES * s.shape[0], *s.shape[1:]), s.dtype), sharding)
        for s in out_avals]
    for a in ds:
        a.block_until_ready()
    return ds


def launch(dev_in, dummies, reps=1):
    jitted, _, _, _, _ = _get_runner(_PLAN, reps)
    return jitted(*dev_in, *dummies)


def assemble(out_arrs):
    out = np.asarray(out_arrs[0]).reshape(NCORES, T, GPC)
    rs = np.arange(G)
    res = np.empty((G, T), np.float32)
    res[_PERM] = out[rs % NCORES, :, rs // NCORES]
    return res


def run_placed(dev_in, dummies=None):
    if dummies is None:
        dummies = make_out_dummies()
    return assemble(launch(dev_in, dummies))


def _prepare_inputs(x, edge_index, edge_attr, batch, num_graphs,
                    emb1, emb2, eemb1, eemb2, W1, b1, W2, b2, bn_g, bn_b,
                    hW1, hb1, hg, hbt, hW2, hb2):
    """Host-side restructuring: receptive-field packing, parameter folding,
    count features, block-diagonal message matrices, shard by graph."""
    global _PERM, _PLAN
    x = np.asarray(x)
    edge_index = np.asarray(edge_index)
    edge_attr = np.asarray(edge_attr)
    fp = lambda a: np.asarray(a, np.float32)
    emb1, emb2 = fp(emb1), fp(emb2)
    eemb1, eemb2 = fp(eemb1), fp(eemb2)
    W1, b1, W2, b2 = fp(W1), fp(b1), fp(W2), fp(b2)
    bn_g, bn_b = fp(bn_g), fp(bn_b)
    hW1, hb1, hg, hbt, hW2, hb2 = (fp(hW1), fp(hb1), fp(hg), fp(hbt),
                                   fp(hW2), fp(hb2))

    bn_inv = np.float32(1.0 / np.sqrt(1.0 + EPS))

    # fold eval-BN into second linear of each GIN MLP
    W2f = W2 * (bn_g * bn_inv)[:, None, :]
    b2f = b2 * (bn_g * bn_inv) + bn_b
    # fold per-layer self-loop constant through W1 into b1
    c = eemb1[:, SELF_LOOP_BOND, :] + eemb2[:, 0, :]            # [L, D]
    b1f = b1 + np.einsum('ld,ldm->lm', c, W1)                   # [L, 2D]

    ecat = np.concatenate([eemb1, eemb2], axis=1)               # [L, 9, D]
    emb0 = np.concatenate([emb1, emb2], axis=0)                 # [124, D]

    src0 = edge_index[0].astype(np.int64)
    dst0 = edge_index[1].astype(np.int64)

    # --- backward receptive fields on the ORIGINAL graph ids ---
    masksL = np.zeros((5, N), bool)
    m = masksL[4]
    m[0::NPG] = True
    for l in (4, 3, 2, 1):
        nm = masksL[l].copy()
        nm[src0[masksL[l][dst0]]] = True
        masksL[l - 1] = nm
    sizes = masksL.reshape(5, G, NPG).sum(2)                    # [5, G]

    # --- sorted round-robin graph placement across cores ---
    # lexicographic by per-layer receptive-field size: positions (= octets of
    # 8 graphs, one per core) get near-identical size profiles at EVERY
    # layer, minimizing the per-position-max padding (~4% vs ~14% for a
    # sum key)
    skey = (sizes[0] * (1 << 30) + sizes[1] * (1 << 20)
            + sizes[2] * (1 << 10) + sizes[3])
    order = np.argsort(-skey, kind="stable")                    # rank -> old g
    core_of = np.arange(G) % NCORES
    pos_of = np.arange(G) // NCORES
    # new node id for (rank r, local j)
    newbase = np.empty(G, np.int64)
    newbase[order] = core_of * NPC + pos_of * NPG
    newid = newbase[np.arange(N) // NPG] + np.arange(N) % NPG
    inv = np.argsort(newid)                                     # new -> old
    _PERM = order

    x_n = x[inv]
    src, dst = newid[src0], newid[dst0]
    maskn = masksL[:, inv]                                      # [5, N] new ids
    # sizes per (l, core, pos)
    s_lcp = maskn.reshape(5, NCORES, GPC, NPG).sum(3)           # [5, 8, 625]
    shat = s_lcp.max(1)                                         # [5, 625]
    plan = _make_plan([tuple(int(v) for v in shat[l]) for l in range(5)])
    off = [np.asarray(o, np.int64) for o in plan["off"]]
    P = plan["P"]

    # packed column index per (l, node): off[l][pos] + rank-in-graph
    ngid = np.arange(N) // NPG                                  # new graph id
    npos = ngid % GPC
    colpos = np.full((5, N), -1, np.int64)
    for l in range(5):
        rk = maskn[l].reshape(G, NPG).cumsum(1).reshape(N) - 1
        sel = maskn[l]
        colpos[l, sel] = off[l][npos[sel]] + rk[sel]

    # rowstart per (l, pos): block start offset in packed l-1 layout
    rowstart = np.zeros((5, GPC), np.int64)
    for l in range(1, 5):
        for t in range(NT):
            for (p0, p1) in plan["blocks"][l][t]:
                rowstart[l, p0:p1] = off[l - 1][p0]

    # F9[v, j] (new ids): incoming bond/direction counts
    F9 = (np.bincount(dst * 9 + edge_attr[:, 0], minlength=N * 9)
          + np.bincount(dst * 9 + 6 + edge_attr[:, 1], minlength=N * 9)
          ).astype(np.float32).reshape(N, 9)

    # layer-0 count features (new ids)
    atom, chir = x_n[:, 0].astype(np.int64), x_n[:, 1].astype(np.int64)
    M124 = (np.bincount(dst * 124 + atom[src], minlength=N * 124)
            + np.bincount(dst * 124 + 120 + chir[src], minlength=N * 124)
            ).astype(np.float32).reshape(N, 124)
    M124[np.arange(N), atom] += 1.0
    M124[np.arange(N), 120 + chir] += 1.0
    M133 = np.concatenate([M124, F9], axis=1)                   # [N, 133]
    TW = (np.concatenate([emb0, ecat[0]], axis=0).astype(np.float64)
          @ W1[0].astype(np.float64)).astype(np.float32)        # [133, 2D]
    twhi_h = np.ascontiguousarray(TW[:128].reshape(128, 8, 128)).astype(_bf16)
    twlo_h = np.ascontiguousarray(TW[128:].reshape(5, 8, 128)).astype(_bf16)

    # shared (replicated) tensors
    w1_h = np.ascontiguousarray(
        W1.reshape(L, 4, 128, 8, 128).transpose(0, 2, 1, 3, 4)).astype(_bf16)
    w2_h = np.ascontiguousarray(
        W2f.reshape(L, 8, 128, 4, 128).transpose(0, 2, 1, 3, 4)).astype(_bf16)
    b1_h = np.ascontiguousarray(b1f.reshape(L, 8, 128).transpose(0, 2, 1))
    b2_h = np.ascontiguousarray(b2f.reshape(L, 4, 128).transpose(0, 2, 1))
    ecat_h = ecat.astype(_bf16)
    hW1s = hW1[:D] + hW1[D:]                                     # [512, 128]
    hw1_h = np.ascontiguousarray(
        hW1s.reshape(4, 128, 128).transpose(1, 0, 2)).astype(_bf16)
    hw2_h = (hW2 * (hg * bn_inv)[:, None]).astype(_bf16)         # [128, T]
    hb2f = (hb2 + hbt @ hW2).reshape(T, 1).astype(np.float32)
    hb1_h = hb1.reshape(128, 1).astype(np.float32)

    # --- per-core bd (block-diagonal + F9 rows) and packed M133 ---
    PB = sum(P[1:])
    core_of_node = np.arange(N) // NPC
    in_maps = []
    for cidx in range(NCORES):
        bdp_c = np.zeros((128, PB), np.float32)
        base = 0
        emask_c = core_of_node[dst] == cidx
        for l in range(1, 5):
            sel = emask_c & maskn[l][dst]
            u, v = src[sel], dst[sel]
            rows = colpos[l - 1][u] - rowstart[l][npos[v]]
            cols = base + colpos[l][v]
            np.add.at(bdp_c, (rows, cols), 1.0)
            # self term
            vs = np.flatnonzero(maskn[l] & (core_of_node == cidx))
            rs = colpos[l - 1][vs] - rowstart[l][npos[vs]]
            cs = base + colpos[l][vs]
            bdp_c[rs, cs] += 1.0
            # F9 rows at partitions 119:128
            bdp_c[np.repeat(np.arange(ROWCAP, 128), len(vs)),
                  np.tile(cs, 9)] = F9[vs].T.reshape(-1)
            base += P[l]
        m133_c = np.zeros((133, P[0]), np.float32)
        vs0 = np.flatnonzero(maskn[0] & (core_of_node == cidx))
        m133_c[:, colpos[0][vs0]] = M133[vs0].T
        in_maps.append(dict(
            mhi=m133_c[:128].astype(_bf16), mlo=m133_c[128:].astype(_bf16),
            twhi=twhi_h, twlo=twlo_h,
            bdp=bdp_c.astype(_bf16),
            w1=w1_h, w2=w2_h, b1=b1_h, b2=b2_h,
            ecat=ecat_h,
            hw1=hw1_h, hw2=hw2_h, hb1=hb1_h, hb2=hb2f,
        ))
    _PLAN = plan
    return in_maps


def kernel(**inputs) -> np.ndarray:
    in_maps = _prepare_inputs(**inputs)
    _get_runner(_PLAN)
    dev_in = place_inputs(in_maps)
    return run_placed(dev_in)
